# revision 13
# baseline (speedup 1.0000x reference)
"""EnergyAttention Trainium2 kernel (8 NeuronCores, head-sharded).

Strategy: shard the 16 heads across 8 cores (2 heads per core). Each core:
  - projects K^T (bf16) and K-natural+ones (bf16) and Q^T (fp32) for its heads
  - runs 5 energy steps with transposed scores S^T[k, q] so softmax's
    k-reduction rides the grad matmul via an appended tens-column
    (ones column scaled by 10 folds step_size=0.1 into the reciprocal)
  - computes its partial output block through its Wo columns (fp32)
Host: transposes/casts inputs, sums the 8 partial outputs.
"""

import numpy as np
import ml_dtypes

BF16 = ml_dtypes.bfloat16

N_CORES = 8
D = 1024
K = 4096
Q = 2048
H = 16
HD = 64
STEPS = 5
STEP_SIZE = 0.1
BETA = 1.0 / np.sqrt(np.float32(HD))  # 1/8

QB = 512

_CACHE = {}


def build_program(d=D, k=K, q=Q, steps=STEPS, n_cores=N_CORES):
    """Build + compile the per-core Bass program. Returns the Bacc object."""
    from contextlib import ExitStack

    import concourse.tile as tile
    from concourse import bacc, mybir

    f32 = mybir.dt.float32
    bf16 = mybir.dt.bfloat16

    ndc = d // 128       # D chunks (contraction for projections)
    nkb = k // 512       # k blocks for K^T projection
    nkc = k // 128       # k chunks for the step loop
    nqb = q // QB        # q blocks
    beta = float(1.0 / np.sqrt(np.float64(HD)))

    nc = bacc.Bacc("TRN2", target_bir_lowering=False, debug=False,
                   num_devices=n_cores)
    ctxT = nc.dram_tensor("ctxT", [d, k], bf16, kind="ExternalInput").ap()
    tgtT = nc.dram_tensor("tgtT", [d, q], f32, kind="ExternalInput").ap()
    wk = nc.dram_tensor("wk", [d, 128], bf16, kind="ExternalInput").ap()
    wq = nc.dram_tensor("wq", [d, 128], f32, kind="ExternalInput").ap()
    woT = nc.dram_tensor("woT", [128, d], f32, kind="ExternalInput").ap()
    out = nc.dram_tensor("out", [q, d], f32, kind="ExternalOutput").ap()

    EXP = mybir.ActivationFunctionType.Exp

    with tile.TileContext(nc) as tc, ExitStack() as ctx:
        # ---------------- persistent pools ----------------
        kt_pool = ctx.enter_context(tc.tile_pool(name="kt", bufs=1))
        kon_pool = ctx.enter_context(tc.tile_pool(name="kones", bufs=1))
        qt_pool = ctx.enter_context(tc.tile_pool(name="qt", bufs=2 * nqb))
        qtb_pool = ctx.enter_context(tc.tile_pool(name="qtb", bufs=2 * nqb))
        w_pool = ctx.enter_context(tc.tile_pool(name="w", bufs=1))

        # per-head padded K^T: other head's rows zeroed -> full-128 contraction
        ktp = [kt_pool.tile([128, k], bf16, tag=f"ktp{h}", name=f"ktp{h}")
               for h in range(2)]
        nc.vector.memset(ktp[0][64:128, :], 0.0)
        nc.vector.memset(ktp[1][0:64, :], 0.0)
        # K natural + tens column, per head: chunk i at cols [65i, 65i+65)
        kones = [kon_pool.tile([128, nkc * 65], bf16, tag=f"kones{h}",
                               name=f"kones{h}")
                 for h in range(2)]
        wk_sb = w_pool.tile([128, d], bf16, tag="wk")
        wq_sb = w_pool.tile([128, d], f32, tag="wq")
        wo_sb = w_pool.tile([128, d], f32, tag="wo")

        for c in range(ndc):
            cs = slice(c * 128, (c + 1) * 128)
            nc.sync.dma_start(out=wk_sb[:, cs], in_=wk[cs, :])
            nc.sync.dma_start(out=wq_sb[:, cs], in_=wq[cs, :])
        nc.sync.dma_start(out=wo_sb[:], in_=woT[:])

        # tens columns: memset whole kones to 10.0; K-nat copies overwrite
        # everything except the per-chunk column 64.
        for h in range(2):
            nc.vector.memset(kones[h][:], 10.0)

        qt_tiles = []
        qtb_tiles = []

        # ---------------- phase A: projections ----------------
        with tc.tile_pool(name="ctxp", bufs=ndc) as ctx_pool, \
             tc.tile_pool(name="tgtp", bufs=ndc) as tgt_pool, \
             tc.tile_pool(name="psA", bufs=2, space="PSUM") as psA, \
             tc.tile_pool(name="psB", bufs=2, space="PSUM") as psB, \
             tc.tile_pool(name="psQ", bufs=2, space="PSUM") as psQ:
            ctx_tiles = [ctx_pool.tile([128, k], bf16, tag="ctx", name=f"ctx{c}")
                         for c in range(ndc)]
            tgt_tiles = [tgt_pool.tile([128, q], f32, tag="tgt", name=f"tgt{c}")
                         for c in range(ndc)]
            for c in range(ndc):
                cs = slice(c * 128, (c + 1) * 128)
                nc.sync.dma_start(out=ctx_tiles[c][:], in_=ctxT[cs, :])
                nc.sync.dma_start(out=tgt_tiles[c][:], in_=tgtT[cs, :])

            # K^T = Wk_pair^T @ context^T  (bf16)
            for kb in range(nkb):
                ks = slice(kb * 512, (kb + 1) * 512)
                pk = psA.tile([128, 512], f32, tag="pk")
                for c in range(ndc):
                    cs = slice(c * 128, (c + 1) * 128)
                    nc.tensor.matmul(out=pk[:], lhsT=wk_sb[:, cs],
                                     rhs=ctx_tiles[c][:, ks],
                                     start=(c == 0), stop=(c == ndc - 1))
                nc.vector.tensor_copy(out=ktp[0][0:64, ks], in_=pk[0:64, :])
                nc.vector.tensor_copy(out=ktp[1][64:128, ks], in_=pk[64:128, :])

            # K natural (both heads side by side), scattered into kones
            for kc in range(nkc):
                ks = slice(kc * 128, (kc + 1) * 128)
                pn = psB.tile([128, 128], f32, tag="pn")
                for c in range(ndc):
                    cs = slice(c * 128, (c + 1) * 128)
                    nc.tensor.matmul(out=pn[:], lhsT=ctx_tiles[c][:, ks],
                                     rhs=wk_sb[:, cs],
                                     start=(c == 0), stop=(c == ndc - 1))
                for h in range(2):
                    nc.vector.tensor_copy(
                        out=kones[h][:, kc * 65:kc * 65 + 64],
                        in_=pn[:, h * 64:(h + 1) * 64])

            # Q^T projection in fp32
            for j in range(nqb):
                qs = slice(j * QB, (j + 1) * QB)
                pq = psQ.tile([128, QB], f32, tag="pq")
                for c in range(ndc):
                    cs = slice(c * 128, (c + 1) * 128)
                    nc.tensor.matmul(out=pq[:], lhsT=wq_sb[:, cs],
                                     rhs=tgt_tiles[c][:, qs],
                                     start=(c == 0), stop=(c == ndc - 1))
                q0 = qt_pool.tile([128, QB], f32, tag="qt")
                nc.vector.tensor_copy(out=q0[:], in_=pq[:])
                qb0 = qtb_pool.tile([128, QB], bf16, tag="qtb")
                nc.vector.tensor_copy(out=qb0[:], in_=q0[:])
                qt_tiles.append(q0)
                qtb_tiles.append(qb0)

        # ---------------- phase B: energy steps ----------------
        # Per (step, q-block, k-chunk): scores S^T via zero-padded per-head
        # K^T (full-128 contraction), one exp over the head pair, grad
        # accumulation through kones (65th column = 10.0 -> denominator).
        with tc.tile_pool(name="pt", bufs=6) as pt_pool, \
             tc.tile_pool(name="upd", bufs=6) as upd_pool, \
             tc.tile_pool(name="ps_s", bufs=3, space="PSUM") as ps_s, \
             tc.tile_pool(name="ps_g", bufs=2, space="PSUM") as ps_g:
            for t in range(steps):
                new_qt = []
                new_qtb = []
                for j in range(nqb):
                    qcur = qt_tiles[j]
                    qbcur = qtb_tiles[j]
                    # one accumulator per head, full-128 contraction
                    gt = [ps_g.tile([65, QB], f32, tag="g", name=f"g{t}_{j}_{i}")
                          for i in range(2)]
                    for kc in range(nkc):
                        s = ps_s.tile([128, 2 * QB], f32, tag="s")
                        for h in range(2):
                            nc.tensor.matmul(
                                out=s[:, h * QB:(h + 1) * QB],
                                lhsT=ktp[h][:, kc * 128:(kc + 1) * 128],
                                rhs=qbcur[:, :],
                                start=True, stop=True)
                        p = pt_pool.tile([128, 2 * QB], bf16, tag="pt")
                        nc.scalar.activation(p[:], s[:], EXP, scale=beta)
                        for h in range(2):
                            nc.tensor.matmul(
                                out=gt[h][:],
                                lhsT=kones[h][:, kc * 65:(kc + 1) * 65],
                                rhs=p[:, h * QB:(h + 1) * QB],
                                start=(kc == 0), stop=(kc == nkc - 1))
                    # q update: q += (G/10) / (denom/10) * 0.1 == q + 0.1*G/denom
                    qn = qt_pool.tile([128, QB], f32, tag="qt")
                    tm = upd_pool.tile([128, QB], f32, tag="tm")
                    for h in range(2):
                        hs = slice(h * 64, (h + 1) * 64)
                        t2 = upd_pool.tile([65, QB], f32, tag="t2")
                        nc.vector.tensor_copy(out=t2[:], in_=gt[h][:])
                        # reciprocal lands on partition 0: partition_broadcast
                        # only reads correctly from a partition-0 source on HW
                        r = upd_pool.tile([1, QB], f32, tag="r")
                        nc.vector.reciprocal(out=r[:], in_=t2[64:65, :])
                        rb = upd_pool.tile([64, QB], f32, tag="rb")
                        nc.gpsimd.partition_broadcast(rb[:], r[0:1, :])
                        nc.vector.tensor_mul(out=tm[hs, :], in0=t2[0:64, :],
                                             in1=rb[:])
                    nc.vector.tensor_add(out=qn[:], in0=qcur[:], in1=tm[:])
                    qb_new = qtb_pool.tile([128, QB], bf16, tag="qtb")
                    nc.vector.tensor_copy(out=qb_new[:], in_=qn[:])
                    new_qt.append(qn)
                    new_qtb.append(qb_new)
                qt_tiles = new_qt
                qtb_tiles = new_qtb

        # ---------------- phase C: output projection (fp32) ----------------
        with tc.tile_pool(name="fo", bufs=3) as fo_pool, \
             tc.tile_pool(name="psO", bufs=2, space="PSUM") as psO:
            dob = min(512, d)
            for qb128 in range(q // 128):
                jt = qt_tiles[(qb128 * 128) // QB]
                qs = slice((qb128 * 128) % QB, (qb128 * 128) % QB + 128)
                for db in range(d // dob):
                    ds_ = slice(db * dob, (db + 1) * dob)
                    po = psO.tile([128, dob], f32, tag="po")
                    nc.tensor.matmul(out=po[:], lhsT=jt[:, qs],
                                     rhs=wo_sb[:, ds_],
                                     start=True, stop=True)
                    ot = fo_pool.tile([128, dob], f32, tag="ot")
                    nc.vector.tensor_copy(out=ot[:], in_=po[:])
                    nc.sync.dma_start(
                        out=out[qb128 * 128:(qb128 + 1) * 128, ds_],
                        in_=ot[:])

    nc.compile()
    return nc


def _get_program():
    if "nc" not in _CACHE:
        _CACHE["nc"] = build_program()
    return _CACHE["nc"]


def make_in_maps(context, target_init, Wq, Wk, Wo):
    """Host-side sharding/layout prep: one input map per core."""
    ctxT = np.ascontiguousarray(context.T).astype(BF16)        # [D, K]
    tgtT = np.ascontiguousarray(target_init.T.astype(np.float32))  # [D, Q]
    in_maps = []
    for c in range(N_CORES):
        h0, h1 = 2 * c, 2 * c + 1
        wk_c = np.concatenate([Wk[h0].T, Wk[h1].T], axis=1)    # [D, 128]
        wq_c = np.concatenate([Wq[h0].T, Wq[h1].T], axis=1)    # [D, 128]
        woT_c = np.ascontiguousarray(Wo[:, 128 * c:128 * (c + 1)].T)  # [128, D]
        in_maps.append({
            "ctxT": ctxT,
            "tgtT": tgtT,
            "wk": np.ascontiguousarray(wk_c).astype(BF16),
            "wq": np.ascontiguousarray(wq_c.astype(np.float32)),
            "woT": woT_c.astype(np.float32),
        })
    return in_maps


def kernel(context, target_init, Wq, Wk, Wo):
    context = np.asarray(context, dtype=np.float32)
    target_init = np.asarray(target_init, dtype=np.float32)
    Wq = np.asarray(Wq, dtype=np.float32)
    Wk = np.asarray(Wk, dtype=np.float32)
    Wo = np.asarray(Wo, dtype=np.float32)

    in_maps = make_in_maps(context, target_init, Wq, Wk, Wo)

    last_err = None
    for _attempt in range(3):
        try:
            results = _run_spmd(in_maps)
            break
        except Exception as e:  # transient axon RESOURCE_EXHAUSTED etc.
            last_err = e
            _CACHE.clear()
    else:
        raise last_err

    acc = np.zeros((Q, D), dtype=np.float32)
    for c in range(N_CORES):
        acc += results[c]["out"]
    return acc


def _run_spmd(in_maps):
    """Run the program on cores 0..7. Uses a cached jitted executable with
    device-resident zero buffers; falls back to run_bass_kernel_spmd."""
    nc = _get_program()
    try:
        runner = _CACHE.get("runner")
        if runner is None:
            runner = _SpmdRunner(nc, N_CORES)
            _CACHE["runner"] = runner
        return runner.run(in_maps)
    except Exception:
        _CACHE.pop("runner", None)
        from concourse.bass_utils import run_bass_kernel_spmd
        res = run_bass_kernel_spmd(nc, in_maps, list(range(N_CORES)))
        return res.results


class _SpmdRunner:
    """Persistent jitted shard_map executable (mirrors
    bass2jax.run_bass_via_pjrt's multi-core path, without output donation so
    the executable and zero buffers are reusable across calls)."""

    def __init__(self, nc, n_cores):
        import jax
        from jax.experimental.shard_map import shard_map
        from jax.sharding import Mesh, NamedSharding, PartitionSpec
        import concourse.mybir as mybir
        from concourse.bass2jax import (
            _bass_exec_p, install_neuronx_cc_hook, partition_id_tensor)

        install_neuronx_cc_hook()
        self.jax = jax
        self.n_cores = n_cores
        partition_name = (nc.partition_id_tensor.name
                          if nc.partition_id_tensor else None)
        in_names, out_names, out_avals, zero_outs = [], [], [], []
        for alloc in nc.m.functions[0].allocations:
            if not isinstance(alloc, mybir.MemoryLocationSet):
                continue
            name = alloc.memorylocations[0].name
            if alloc.kind == "ExternalInput":
                if name != partition_name:
                    in_names.append(name)
            elif alloc.kind == "ExternalOutput":
                shape = tuple(alloc.tensor_shape)
                dtype = mybir.dt.np(alloc.dtype)
                out_names.append(name)
                out_avals.append(jax.core.ShapedArray(shape, dtype))
                zero_outs.append(np.zeros(shape, dtype))
        self.in_names = in_names
        self.out_names = out_names
        self.out_avals = out_avals
        all_in_names = in_names + out_names
        if partition_name is not None:
            all_in_names.append(partition_name)

        def _body(*args):
            operands = list(args)
            if partition_name is not None:
                operands.append(partition_id_tensor())
            outs = _bass_exec_p.bind(
                *operands,
                out_avals=tuple(out_avals),
                in_names=tuple(all_in_names),
                out_names=tuple(out_names),
                lowering_input_output_aliases=(),
                sim_require_finite=True,
                sim_require_nnan=True,
                nc=nc,
            )
            return tuple(outs)

        devices = jax.devices()[:n_cores]
        mesh = Mesh(np.asarray(devices), ("core",))
        in_specs = (PartitionSpec("core"),) * (len(in_names) + len(out_names))
        out_specs = (PartitionSpec("core"),) * len(out_names)
        self.fn = jax.jit(
            shard_map(_body, mesh=mesh, in_specs=in_specs,
                      out_specs=out_specs, check_rep=False),
            keep_unused=True,
        )
        self.sharding = NamedSharding(mesh, PartitionSpec("core"))
        self.zeros_placed = [
            jax.device_put(np.concatenate([z] * n_cores, axis=0), self.sharding)
            for z in zero_outs
        ]

    def place(self, in_maps):
        concat = [
            np.concatenate([np.asarray(in_maps[c][n])
                            for c in range(self.n_cores)], axis=0)
            for n in self.in_names
        ]
        return [self.jax.device_put(a, self.sharding) for a in concat]

    def exec_placed(self, placed):
        outs = self.fn(*placed, *self.zeros_placed)
        self.jax.block_until_ready(outs)
        return outs

    def run(self, in_maps):
        outs = self.exec_placed(self.place(in_maps))
        per_core = []
        for c in range(self.n_cores):
            d = {}
            for i, n in enumerate(self.out_names):
                full = np.asarray(outs[i])
                sh = self.out_avals[i].shape
                d[n] = full.reshape(self.n_cores, *sh)[c]
            per_core.append(d)
        return per_core


# revision 17
# speedup vs baseline: 1.0643x; 1.0643x over previous
"""EnergyAttention Trainium2 kernel (8 NeuronCores, head-sharded).

Strategy: shard the 16 heads across 8 cores (2 heads per core). Each core:
  - projects K^T (bf16) and K-natural+ones (bf16) and Q^T (fp32) for its heads
  - runs 5 energy steps with transposed scores S^T[k, q] so softmax's
    k-reduction rides the grad matmul via an appended tens-column
    (ones column scaled by 10 folds step_size=0.1 into the reciprocal)
  - computes its partial output block through its Wo columns (fp32)
Host: transposes/casts inputs, sums the 8 partial outputs.
"""

import numpy as np
import ml_dtypes

BF16 = ml_dtypes.bfloat16

N_CORES = 8
D = 1024
K = 4096
Q = 2048
H = 16
HD = 64
STEPS = 5
STEP_SIZE = 0.1
BETA = 1.0 / np.sqrt(np.float32(HD))  # 1/8

QB = 512

_CACHE = {}


def build_program(d=D, k=K, q=Q, steps=STEPS, n_cores=N_CORES):
    """Build + compile the per-core Bass program. Returns the Bacc object."""
    from contextlib import ExitStack

    import concourse.tile as tile
    from concourse import bacc, mybir

    f32 = mybir.dt.float32
    bf16 = mybir.dt.bfloat16

    ndc = d // 128       # D chunks (contraction for projections)
    nkb = k // 512       # k blocks for K^T projection
    nkc = k // 128       # k chunks for the step loop
    nqb = q // QB        # q blocks
    beta = float(1.0 / np.sqrt(np.float64(HD)))

    nc = bacc.Bacc("TRN2", target_bir_lowering=False, debug=False,
                   num_devices=n_cores)
    ctxT = nc.dram_tensor("ctxT", [d, k], bf16, kind="ExternalInput").ap()
    tgtT = nc.dram_tensor("tgtT", [d, q], f32, kind="ExternalInput").ap()
    wk = nc.dram_tensor("wk", [d, 128], bf16, kind="ExternalInput").ap()
    wq = nc.dram_tensor("wq", [d, 128], f32, kind="ExternalInput").ap()
    woT = nc.dram_tensor("woT", [128, d], f32, kind="ExternalInput").ap()
    out = nc.dram_tensor("out", [q, d], f32, kind="ExternalOutput").ap()

    EXP = mybir.ActivationFunctionType.Exp

    with tile.TileContext(nc) as tc, ExitStack() as ctx:
        # ---------------- persistent pools ----------------
        kt_pool = ctx.enter_context(tc.tile_pool(name="kt", bufs=1))
        kon_pool = ctx.enter_context(tc.tile_pool(name="kones", bufs=1))
        qt_pool = ctx.enter_context(tc.tile_pool(name="qt", bufs=2 * nqb))
        qtb_pool = ctx.enter_context(tc.tile_pool(name="qtb", bufs=2 * nqb))
        w_pool = ctx.enter_context(tc.tile_pool(name="w", bufs=1))

        # per-head padded K^T: other head's rows zeroed -> full-128 contraction
        ktp = [kt_pool.tile([128, k], bf16, tag=f"ktp{h}", name=f"ktp{h}")
               for h in range(2)]
        nc.vector.memset(ktp[0][64:128, :], 0.0)
        nc.vector.memset(ktp[1][0:64, :], 0.0)
        # K natural + tens column, per head: chunk i at cols [65i, 65i+65)
        kones = [kon_pool.tile([128, nkc * 65], bf16, tag=f"kones{h}",
                               name=f"kones{h}")
                 for h in range(2)]
        wk_sb = w_pool.tile([128, d], bf16, tag="wk")
        wq_sb = w_pool.tile([128, d], f32, tag="wq")
        wo_sb = w_pool.tile([128, d], f32, tag="wo")

        for c in range(ndc):
            cs = slice(c * 128, (c + 1) * 128)
            nc.sync.dma_start(out=wk_sb[:, cs], in_=wk[cs, :])
            nc.sync.dma_start(out=wq_sb[:, cs], in_=wq[cs, :])
        nc.sync.dma_start(out=wo_sb[:], in_=woT[:])

        # tens columns: memset whole kones to 10.0; K-nat copies overwrite
        # everything except the per-chunk column 64.
        for h in range(2):
            nc.vector.memset(kones[h][:], 10.0)

        qt_tiles = []
        qtb_tiles = []

        # ---------------- phase A: projections ----------------
        with tc.tile_pool(name="ctxp", bufs=ndc) as ctx_pool, \
             tc.tile_pool(name="tgtp", bufs=ndc) as tgt_pool, \
             tc.tile_pool(name="psA", bufs=2, space="PSUM") as psA, \
             tc.tile_pool(name="psB", bufs=2, space="PSUM") as psB, \
             tc.tile_pool(name="psQ", bufs=2, space="PSUM") as psQ:
            ctx_tiles = [ctx_pool.tile([128, k], bf16, tag="ctx", name=f"ctx{c}")
                         for c in range(ndc)]
            tgt_tiles = [tgt_pool.tile([128, q], f32, tag="tgt", name=f"tgt{c}")
                         for c in range(ndc)]
            for c in range(ndc):
                cs = slice(c * 128, (c + 1) * 128)
                nc.sync.dma_start(out=ctx_tiles[c][:], in_=ctxT[cs, :])
                nc.sync.dma_start(out=tgt_tiles[c][:], in_=tgtT[cs, :])

            # K^T = Wk_pair^T @ context^T  (bf16)
            for kb in range(nkb):
                ks = slice(kb * 512, (kb + 1) * 512)
                pk = psA.tile([128, 512], f32, tag="pk")
                for c in range(ndc):
                    cs = slice(c * 128, (c + 1) * 128)
                    nc.tensor.matmul(out=pk[:], lhsT=wk_sb[:, cs],
                                     rhs=ctx_tiles[c][:, ks],
                                     start=(c == 0), stop=(c == ndc - 1))
                nc.vector.tensor_copy(out=ktp[0][0:64, ks], in_=pk[0:64, :])
                nc.vector.tensor_copy(out=ktp[1][64:128, ks], in_=pk[64:128, :])

            # K natural (both heads side by side), scattered into kones
            for kc in range(nkc):
                ks = slice(kc * 128, (kc + 1) * 128)
                pn = psB.tile([128, 128], f32, tag="pn")
                for c in range(ndc):
                    cs = slice(c * 128, (c + 1) * 128)
                    nc.tensor.matmul(out=pn[:], lhsT=ctx_tiles[c][:, ks],
                                     rhs=wk_sb[:, cs],
                                     start=(c == 0), stop=(c == ndc - 1))
                for h in range(2):
                    nc.vector.tensor_copy(
                        out=kones[h][:, kc * 65:kc * 65 + 64],
                        in_=pn[:, h * 64:(h + 1) * 64])

            # Q^T projection in fp32
            for j in range(nqb):
                qs = slice(j * QB, (j + 1) * QB)
                pq = psQ.tile([128, QB], f32, tag="pq")
                for c in range(ndc):
                    cs = slice(c * 128, (c + 1) * 128)
                    nc.tensor.matmul(out=pq[:], lhsT=wq_sb[:, cs],
                                     rhs=tgt_tiles[c][:, qs],
                                     start=(c == 0), stop=(c == ndc - 1))
                q0 = qt_pool.tile([128, QB], f32, tag="qt")
                nc.vector.tensor_copy(out=q0[:], in_=pq[:])
                qb0 = qtb_pool.tile([128, QB], bf16, tag="qtb")
                nc.vector.tensor_copy(out=qb0[:], in_=q0[:])
                qt_tiles.append(q0)
                qtb_tiles.append(qb0)

        # ---------------- phase B: energy steps ----------------
        # Per (step, q-block, k-chunk): scores S^T via zero-padded per-head
        # K^T (full-128 contraction), one exp over the head pair, grad
        # accumulation through kones (65th column = 10.0 -> denominator).
        with tc.tile_pool(name="pt", bufs=6) as pt_pool, \
             tc.tile_pool(name="upd", bufs=6) as upd_pool, \
             tc.tile_pool(name="ps_s", bufs=3, space="PSUM") as ps_s, \
             tc.tile_pool(name="ps_g", bufs=2, space="PSUM") as ps_g:
            for t in range(steps):
                new_qt = []
                new_qtb = []
                for j in range(nqb):
                    qcur = qt_tiles[j]
                    qbcur = qtb_tiles[j]
                    # one accumulator per head, full-128 contraction
                    gt = [ps_g.tile([65, QB], f32, tag="g", name=f"g{t}_{j}_{i}")
                          for i in range(2)]
                    for kc in range(nkc):
                        s = ps_s.tile([128, 2 * QB], f32, tag="s")
                        for h in range(2):
                            nc.tensor.matmul(
                                out=s[:, h * QB:(h + 1) * QB],
                                lhsT=ktp[h][:, kc * 128:(kc + 1) * 128],
                                rhs=qbcur[:, :],
                                start=True, stop=True)
                        p = pt_pool.tile([128, 2 * QB], bf16, tag="pt")
                        nc.scalar.activation(p[:], s[:], EXP, scale=beta)
                        for h in range(2):
                            nc.tensor.matmul(
                                out=gt[h][:],
                                lhsT=kones[h][:, kc * 65:(kc + 1) * 65],
                                rhs=p[:, h * QB:(h + 1) * QB],
                                start=(kc == 0), stop=(kc == nkc - 1))
                    # q update: q += (G/10) / (denom/10) * 0.1 == q + 0.1*G/denom
                    qn = qt_pool.tile([128, QB], f32, tag="qt")
                    tm = upd_pool.tile([128, QB], f32, tag="tm")
                    for h in range(2):
                        hs = slice(h * 64, (h + 1) * 64)
                        t2 = upd_pool.tile([65, QB], f32, tag="t2")
                        nc.vector.tensor_copy(out=t2[:], in_=gt[h][:])
                        # reciprocal lands on partition 0: partition_broadcast
                        # only reads correctly from a partition-0 source on HW
                        r = upd_pool.tile([1, QB], f32, tag="r")
                        nc.vector.reciprocal(out=r[:], in_=t2[64:65, :])
                        rb = upd_pool.tile([64, QB], f32, tag="rb")
                        nc.gpsimd.partition_broadcast(rb[:], r[0:1, :])
                        nc.vector.tensor_mul(out=tm[hs, :], in0=t2[0:64, :],
                                             in1=rb[:])
                    nc.vector.tensor_add(out=qn[:], in0=qcur[:], in1=tm[:])
                    qb_new = qtb_pool.tile([128, QB], bf16, tag="qtb")
                    nc.vector.tensor_copy(out=qb_new[:], in_=qn[:])
                    new_qt.append(qn)
                    new_qtb.append(qb_new)
                qt_tiles = new_qt
                qtb_tiles = new_qtb

        # ---------------- phase C: output projection (fp32) ----------------
        with tc.tile_pool(name="fo", bufs=3) as fo_pool, \
             tc.tile_pool(name="psO", bufs=2, space="PSUM") as psO:
            dob = min(512, d)
            for qb128 in range(q // 128):
                jt = qt_tiles[(qb128 * 128) // QB]
                qs = slice((qb128 * 128) % QB, (qb128 * 128) % QB + 128)
                for db in range(d // dob):
                    ds_ = slice(db * dob, (db + 1) * dob)
                    po = psO.tile([128, dob], f32, tag="po")
                    nc.tensor.matmul(out=po[:], lhsT=jt[:, qs],
                                     rhs=wo_sb[:, ds_],
                                     start=True, stop=True)
                    ot = fo_pool.tile([128, dob], f32, tag="ot")
                    nc.vector.tensor_copy(out=ot[:], in_=po[:])
                    nc.sync.dma_start(
                        out=out[qb128 * 128:(qb128 + 1) * 128, ds_],
                        in_=ot[:])

    nc.compile()
    return nc


def _get_program():
    if "nc" not in _CACHE:
        _CACHE["nc"] = build_program()
    return _CACHE["nc"]


def make_in_maps(context, target_init, Wq, Wk, Wo):
    """Host-side sharding/layout prep: one input map per core."""
    ctxT = np.ascontiguousarray(context.T).astype(BF16)        # [D, K]
    tgtT = np.ascontiguousarray(target_init.T.astype(np.float32))  # [D, Q]
    in_maps = []
    for c in range(N_CORES):
        h0, h1 = 2 * c, 2 * c + 1
        wk_c = np.concatenate([Wk[h0].T, Wk[h1].T], axis=1)    # [D, 128]
        wq_c = np.concatenate([Wq[h0].T, Wq[h1].T], axis=1)    # [D, 128]
        woT_c = np.ascontiguousarray(Wo[:, 128 * c:128 * (c + 1)].T)  # [128, D]
        in_maps.append({
            "ctxT": ctxT,
            "tgtT": tgtT,
            "wk": np.ascontiguousarray(wk_c).astype(BF16),
            "wq": np.ascontiguousarray(wq_c.astype(np.float32)),
            "woT": woT_c.astype(np.float32),
        })
    return in_maps


def kernel(context, target_init, Wq, Wk, Wo):
    context = np.asarray(context, dtype=np.float32)
    target_init = np.asarray(target_init, dtype=np.float32)
    Wq = np.asarray(Wq, dtype=np.float32)
    Wk = np.asarray(Wk, dtype=np.float32)
    Wo = np.asarray(Wo, dtype=np.float32)

    in_maps = make_in_maps(context, target_init, Wq, Wk, Wo)

    last_err = None
    for _attempt in range(3):
        try:
            results = _run_spmd(in_maps)
            break
        except Exception as e:  # transient axon RESOURCE_EXHAUSTED etc.
            last_err = e
            _CACHE.clear()
    else:
        raise last_err

    acc = np.zeros((Q, D), dtype=np.float32)
    for c in range(N_CORES):
        acc += results[c]["out"]
    return acc


def _run_spmd(in_maps):
    """Run the program on cores 0..7. Uses a cached jitted executable with
    device-resident zero buffers; falls back to run_bass_kernel_spmd."""
    nc = _get_program()
    try:
        runner = _CACHE.get("runner")
        if runner is None:
            runner = _SpmdRunner(nc, N_CORES)
            _CACHE["runner"] = runner
        return runner.run(in_maps)
    except Exception:
        _CACHE.pop("runner", None)
        from concourse.bass_utils import run_bass_kernel_spmd
        res = run_bass_kernel_spmd(nc, in_maps, list(range(N_CORES)))
        return res.results


class _SpmdRunner:
    """Persistent jitted shard_map executable (mirrors
    bass2jax.run_bass_via_pjrt's multi-core path, without output donation so
    the executable and zero buffers are reusable across calls)."""

    def __init__(self, nc, n_cores):
        import jax
        from jax.experimental.shard_map import shard_map
        from jax.sharding import Mesh, NamedSharding, PartitionSpec
        import concourse.mybir as mybir
        from concourse.bass2jax import (
            _bass_exec_p, install_neuronx_cc_hook, partition_id_tensor)

        install_neuronx_cc_hook()
        self.jax = jax
        self.n_cores = n_cores
        partition_name = (nc.partition_id_tensor.name
                          if nc.partition_id_tensor else None)
        in_names, out_names, out_avals, zero_outs = [], [], [], []
        for alloc in nc.m.functions[0].allocations:
            if not isinstance(alloc, mybir.MemoryLocationSet):
                continue
            name = alloc.memorylocations[0].name
            if alloc.kind == "ExternalInput":
                if name != partition_name:
                    in_names.append(name)
            elif alloc.kind == "ExternalOutput":
                shape = tuple(alloc.tensor_shape)
                dtype = mybir.dt.np(alloc.dtype)
                out_names.append(name)
                out_avals.append(jax.core.ShapedArray(shape, dtype))
                zero_outs.append(np.zeros(shape, dtype))
        self.in_names = in_names
        self.out_names = out_names
        self.out_avals = out_avals
        all_in_names = in_names + out_names
        if partition_name is not None:
            all_in_names.append(partition_name)

        def _body(*args):
            operands = list(args)
            if partition_name is not None:
                operands.append(partition_id_tensor())
            outs = _bass_exec_p.bind(
                *operands,
                out_avals=tuple(out_avals),
                in_names=tuple(all_in_names),
                out_names=tuple(out_names),
                lowering_input_output_aliases=(),
                sim_require_finite=True,
                sim_require_nnan=True,
                nc=nc,
            )
            return tuple(outs)

        devices = jax.devices()[:n_cores]
        mesh = Mesh(np.asarray(devices), ("core",))
        in_specs = (PartitionSpec("core"),) * (len(in_names) + len(out_names))
        out_specs = (PartitionSpec("core"),) * len(out_names)
        self.fn = jax.jit(
            shard_map(_body, mesh=mesh, in_specs=in_specs,
                      out_specs=out_specs, check_rep=False),
            keep_unused=True,
        )
        self.sharding = NamedSharding(mesh, PartitionSpec("core"))
        self.zeros_placed = [
            jax.device_put(np.concatenate([z] * n_cores, axis=0), self.sharding)
            for z in zero_outs
        ]

    def place(self, in_maps):
        concat = [
            np.concatenate([np.asarray(in_maps[c][n])
                            for c in range(self.n_cores)], axis=0)
            for n in self.in_names
        ]
        return [self.jax.device_put(a, self.sharding) for a in concat]

    def exec_placed(self, placed):
        outs = self.fn(*placed, *self.zeros_placed)
        self.jax.block_until_ready(outs)
        return outs

    def run(self, in_maps):
        outs = self.exec_placed(self.place(in_maps))
        per_core = []
        for c in range(self.n_cores):
            d = {}
            for i, n in enumerate(self.out_names):
                full = np.asarray(outs[i])
                sh = self.out_avals[i].shape
                d[n] = full.reshape(self.n_cores, *sh)[c]
            per_core.append(d)
        return per_core
